# revision 18
# baseline (speedup 1.0000x reference)
"""DAWN moe_routing kernel for 8 trn2 NeuronCores.

Sharding:
  - 4 transformer layers: data-parallel over tokens, 8 shards of 512 tokens
    (batch b = core//2, sequence half h = core%2). Causal attention needs the
    other half's activations: each core layer-norms its own tokens and the
    pair exchanges the *normalized* activations (bf16, 0.5MB) with a pair-wise
    AllGather. Pair buffers are own-tokens-first so the causal mask structure
    is identical on every core; the other-half key blocks are gated by a
    per-core exp bias (0 for the second-half core, -30 for the first-half).
  - tied head: vocab-sharded (4000 columns per core) after an 8-way AllGather
    of the final activations (bf16); each core computes logits for all 4096
    tokens x its own vocab shard, starting with its own token shard so the
    AllGather latency is hidden. Logits are written fp16.
Activations are kept d-major (transposed) end to end: layer norm runs via
ones-matmul column reductions + K=1 broadcast matmuls, so no PE transposes
are needed for x / residual / context. All big matmuls run bf16 (fp32 PSUM
accumulation); the residual stream stays fp32 in SBUF.
"""
import math
from contextlib import ExitStack

import numpy as np

import concourse.bass as bass
import concourse.bacc as bacc
import concourse.mybir as mybir
import concourse.tile as tile

B, S, D, H, KSEL, L = 4, 1024, 512, 8, 8, 4
VOCAB, DFF, NB, RANK, NNEU = 32000, 2048, 32, 64, 64
NC = 8
T = 512            # local tokens per core
PAIR = 1024
DH = 64
NR = NB * RANK     # 2048
VSH = VOCAB // NC  # 4000
TT = T // 128      # 4
DC = D // 128      # 4
FC = DFF // 128    # 16
CC = NR // 128     # 16
f32 = mybir.dt.float32
f32r = mybir.dt.float32r
bf16 = mybir.dt.bfloat16
fp16 = mybir.dt.float16
AF = mybir.ActivationFunctionType
ALU = mybir.AluOpType
AX = mybir.AxisListType
EPS = 1e-5
NEG = -30.0
VWID = [512] * 7 + [VSH - 7 * 512]  # head vocab block widths


def mmr(nc, out, lhsT, rhs, start, stop):
    nc.tensor.matmul(out, lhsT, rhs, start=start, stop=stop)


def _cp(eng, dst, src):
    if eng.engine == mybir.EngineType.DVE:
        eng.tensor_copy(dst, src)
    else:
        eng.copy(dst, src)


def build_nc(gelu_af=AF.Gelu, reps=1):
    nc = bacc.Bacc("TRN2", target_bir_lowering=False, debug=False, num_devices=NC)

    x0_d = nc.dram_tensor("x0T", [D, T], f32r, kind="ExternalInput")
    wq_d = nc.dram_tensor("Wqb", [L, D, D], bf16, kind="ExternalInput")
    wk_d = nc.dram_tensor("Wkb", [L, D, D], bf16, kind="ExternalInput")
    wv_d = nc.dram_tensor("Wvb", [L, D, D], bf16, kind="ExternalInput")
    ws_d = nc.dram_tensor("Wsb", [L, 2 * D, D], bf16, kind="ExternalInput")
    wd_d = nc.dram_tensor("Wdbf", [L, DFF, D], bf16, kind="ExternalInput")
    # Abf[d, r*NB + n] = basis_A[n, d, r]  (r-major for contiguous reduce)
    af_d = nc.dram_tensor("Abf", [D, NR], bf16, kind="ExternalInput")
    # BBbf[f, p, c*128+j] = basis_B_flat[c*128+p, f*128+j]
    bb_d = nc.dram_tensor("BBbf", [FC, 128, NR], bf16, kind="ExternalInput")
    rec_d = nc.dram_tensor("recipe", [L, NNEU, NB], f32, kind="ExternalInput")
    bemb_d = nc.dram_tensor("bemb", [NB, D], f32, kind="ExternalInput")
    ect_d = nc.dram_tensor("EcTb", [D, VSH], bf16, kind="ExternalInput")
    id_d = nc.dram_tensor("ident", [128, 128], f32, kind="ExternalInput")
    tri_d = nc.dram_tensor("trib", [128, 128], bf16, kind="ExternalInput")
    bg_d = nc.dram_tensor("biasg", [128, 1], f32, kind="ExternalInput")
    sel_d = nc.dram_tensor("selb", [NB, CC * 128], bf16, kind="ExternalInput")
    selq_d = nc.dram_tensor("selq", [64, 128], f32r, kind="ExternalInput")
    onc_d = nc.dram_tensor("onescol", [128, 1], f32r, kind="ExternalInput")
    onr_d = nc.dram_tensor("onesrow", [1, 128], f32r, kind="ExternalInput")
    out_d = nc.dram_tensor("logits", [NC * T, VSH], fp16, kind="ExternalOutput")

    with tile.TileContext(nc) as tc:
        with ExitStack() as _st0:
            cpool = _st0.enter_context(tc.tile_pool(name="const", bufs=1))
            dpool = _st0.enter_context(tc.tile_pool(name="dram", bufs=2, space="DRAM"))
            ppool = _st0.enter_context(tc.tile_pool(name="persist", bufs=1))
            ident = cpool.tile([128, 128], f32, tag="ident")
            nc.sync.dma_start(ident, id_d[:, :])
            tri = cpool.tile([128, 128], bf16, tag="tri")
            nc.sync.dma_start(tri, tri_d[:, :])
            biasg = cpool.tile([128, 1], f32, tag="biasg")
            nc.sync.dma_start(biasg, bg_d[:, :])
            selb = cpool.tile([NB, CC * 128], bf16, tag="selb")
            nc.sync.dma_start(selb, sel_d[:, :])
            selq = cpool.tile([64, 128], f32r, tag="selq")
            nc.sync.dma_start(selq, selq_d[:, :])
            onesc = cpool.tile([128, 1], f32r, tag="onesc")
            nc.sync.dma_start(onesc, onc_d[:, :])
            onesr = cpool.tile([1, 128], f32r, tag="onesr")
            nc.sync.dma_start(onesr, onr_d[:, :])
            epst = cpool.tile([1, 1], f32, tag="epst")
            nc.vector.memset(epst, EPS)

            pid = nc.partition_id()
            offp = (1 - (pid & 1)) * D  # partner block row offset in pair gather

            for _rep in range(reps):
                xl = ppool.tile([128, DC * T], f32r, tag="xl")  # d-major residual
                nc.sync.dma_start(
                    xl.rearrange("p (c t) -> p c t", c=DC),
                    x0_d.rearrange("(c p) t -> p c t", p=128),
                )
                af_bf = ppool.tile([128, DC * NR], bf16, tag="af_bf")
                nc.sync.dma_start(
                    af_bf.rearrange("p (c n) -> p c n", c=DC),
                    af_d.rearrange("(c p) n -> p c n", p=128),
                )
                v65 = ppool.tile([128, 8 * 520], bf16, tag="v65")
                nc.vector.memset(v65, 1.0)
                xfT = ppool.tile([128, DC * T], bf16, tag="xfT")

                with ExitStack() as _st1:
                    bigp = _st1.enter_context(tc.tile_pool(name="big", bufs=1))
                    medp = _st1.enter_context(tc.tile_pool(name="med", bufs=1))
                    w1pool = _st1.enter_context(tc.tile_pool(name="w1", bufs=1))
                    tmpool = _st1.enter_context(tc.tile_pool(name="tm2", bufs=2))
                    strp = _st1.enter_context(tc.tile_pool(name="stream", bufs=2))
                    str3 = _st1.enter_context(tc.tile_pool(name="str3", bufs=3))
                    bb2p = _st1.enter_context(tc.tile_pool(name="bb2", bufs=2))
                    smallp = _st1.enter_context(tc.tile_pool(name="small", bufs=2))
                    s1p = _st1.enter_context(tc.tile_pool(name="small1", bufs=1))
                    psA = _st1.enter_context(tc.tile_pool(name="psA", bufs=4, space="PSUM"))
                    psS = _st1.enter_context(tc.tile_pool(name="psS", bufs=2, space="PSUM"))
                    psC = _st1.enter_context(tc.tile_pool(name="psC", bufs=1, space="PSUM"))
                    psT = _st1.enter_context(tc.tile_pool(name="psT", bufs=1, space="PSUM"))
                    def layer_norm_T(xsrc_cols, outT, out_step):
                        """d-major LN over T tokens; writes bf16 into outT col dc*out_step."""
                        mu_ps = psS.tile([128, T], f32, tag="ps")
                        for dc in range(DC):
                            mmr(nc, mu_ps[0:1, :], onesc, xsrc_cols(dc),
                                start=(dc == 0), stop=(dc == DC - 1))
                        s2_ps = psS.tile([128, T], f32, tag="ps")
                        for dc in range(DC):
                            sq = strp.tile([128, T], f32r, tag="lnsq")
                            nc.scalar.activation(sq, xsrc_cols(dc), AF.Square)
                            mmr(nc, s2_ps[0:1, :], onesc, sq,
                                start=(dc == 0), stop=(dc == DC - 1))
                        mu = s1p.tile([1, T], f32, tag="lnmu")
                        nc.vector.tensor_scalar(mu, mu_ps[0:1, :], 1.0 / D, None, op0=ALU.mult)
                        s2n = s1p.tile([1, T], f32, tag="lns2")
                        nc.vector.tensor_scalar(s2n, s2_ps[0:1, :], 1.0 / D, None, op0=ALU.mult)
                        var = s1p.tile([1, T], f32, tag="lnvar")
                        # var = mu*mu - s2n  (negated variance)
                        nc.vector.tensor_tensor(var, mu, mu, op=ALU.mult)
                        nc.vector.tensor_tensor(var, var, s2n, op=ALU.subtract)
                        sd = s1p.tile([1, T], f32, tag="lnsd")
                        nc.scalar.activation(sd, var, AF.Sqrt, bias=epst[0:1, 0:1], scale=-1.0)
                        rs = s1p.tile([1, T], f32r, tag="lnrs")
                        with nc.allow_low_precision(reason="f32r reciprocal keeps full fp32 bits"):
                            nc.vector.reciprocal(rs, sd)
                        msc = s1p.tile([1, T], f32r, tag="lnmsc")
                        nc.vector.tensor_tensor(msc, mu, rs, op=ALU.mult)
                        rs_ps = psA.tile([128, T], f32, tag="pa")
                        mmr(nc, rs_ps, onesr, rs, start=True, stop=True)
                        ms_ps = psA.tile([128, T], f32, tag="pa")
                        mmr(nc, ms_ps, onesr, msc, start=True, stop=True)
                        for dc in range(DC):
                            tmp = strp.tile([128, T], f32, tag="lntmp")
                            nc.vector.tensor_tensor(tmp, xsrc_cols(dc), rs_ps, op=ALU.mult)
                            nc.vector.tensor_tensor(
                                outT[:, dc * out_step : dc * out_step + T],
                                tmp, ms_ps, op=ALU.subtract)

                    for l in range(L):
                        # ---------- per-layer weight loads ----------
                        wq_t = w1pool.tile([128, DC * D], bf16, tag="wq")
                        nc.sync.dma_start(
                            wq_t.rearrange("p (k n) -> p k n", k=DC),
                            wq_d[l].rearrange("(k p) n -> p k n", p=128),
                        )
                        wk_t = w1pool.tile([128, DC * D], bf16, tag="wk")
                        nc.sync.dma_start(
                            wk_t.rearrange("p (k n) -> p k n", k=DC),
                            wk_d[l].rearrange("(k p) n -> p k n", p=128),
                        )
                        wv_t = w1pool.tile([128, DC * D], bf16, tag="wv")
                        nc.sync.dma_start(
                            wv_t.rearrange("p (k n) -> p k n", k=DC),
                            wv_d[l].rearrange("(k p) n -> p k n", p=128),
                        )
                        ws_t = w1pool.tile([128, 2 * DC * D], bf16, tag="ws")
                        nc.sync.dma_start(
                            ws_t.rearrange("p (k n) -> p k n", k=2 * DC),
                            ws_d[l].rearrange("(k p) n -> p k n", p=128),
                        )
                        wd_t = w1pool.tile([128, FC * D], bf16, tag="wd")
                        nc.sync.dma_start(
                            wd_t.rearrange("p (f n) -> p f n", f=FC),
                            wd_d[l].rearrange("(f p) n -> p f n", p=128),
                        )

                        # ---------- router constants (independent of x) ----------
                        recs = smallp.tile([NNEU, NB], f32, tag="recs")
                        nc.sync.dma_start(recs, rec_d[l, :, :])
                        pe_exp = smallp.tile([NNEU, NB], f32, tag="pe_exp")
                        pe_sum = smallp.tile([NNEU, 1], f32, tag="pe_sum")
                        nc.scalar.activation(pe_exp, recs, AF.Exp, accum_out=pe_sum)
                        pe_inv = smallp.tile([NNEU, 1], f32, tag="pe_inv")
                        nc.vector.reciprocal(pe_inv, pe_sum)
                        pmat = smallp.tile([NNEU, NB], f32, tag="pmat")
                        nc.vector.tensor_scalar(pmat, pe_exp, pe_inv, None, op0=ALU.mult)
                        pt_ps = psT.tile([128, 128], f32, tag="pt")
                        nc.tensor.transpose(pt_ps[0:NB, 0:NNEU], pmat, ident[0:NNEU, 0:NNEU])
                        ptm = smallp.tile([NB, NNEU], f32, tag="ptm")
                        nc.scalar.copy(ptm, pt_ps[0:NB, 0:NNEU])
                        bembs = s1p.tile([NB, D], f32, tag="bembs")
                        nc.sync.dma_start(bembs, bemb_d[:, :])
                        nembT = s1p.tile([128, DC * NNEU], bf16, tag="nembT")
                        for dc in range(DC):
                            ne_ps = psT.tile([128, 128], f32, tag="pt")
                            nc.tensor.matmul(
                                ne_ps[:, 0:NNEU], bembs[:, dc * 128 : (dc + 1) * 128], ptm,
                                start=True, stop=True,
                            )
                            nc.scalar.copy(nembT[:, dc * NNEU : (dc + 1) * NNEU], ne_ps[:, 0:NNEU])

                        # ---------- LN1 own tokens (d-major) + pair exchange ----------
                        n1T = bigp.tile([128, DC * PAIR], bf16, tag="n1T")
                        layer_norm_T(lambda dc: xl[:, dc * T : (dc + 1) * T], n1T, PAIR)
                        n1b = dpool.tile([D, T], bf16, tag="n1b")
                        nc.sync.dma_start(
                            n1b.rearrange("(c p) t -> p c t", p=128),
                            n1T.rearrange("p (c t) -> p c t", c=DC)[:, :, 0:T],
                        )
                        n1p = dpool.tile([2 * D, T], bf16, tag="n1p")
                        nc.gpsimd.collective_compute(
                            "AllGather",
                            ALU.bypass,
                            replica_groups=[[0, 1], [2, 3], [4, 5], [6, 7]],
                            ins=[n1b.opt()],
                            outs=[n1p.opt()],
                        )

                        # ---------- LN2 (d-major, own) ----------
                        n2T = medp.tile([128, DC * T], bf16, tag="n2T")
                        layer_norm_T(lambda dc: xl[:, dc * T : (dc + 1) * T], n2T, T)

                        # ---------- q projection (own half only) ----------
                        qT = medp.tile([128, DC * T], bf16, tag="qT")
                        for mc in range(DC):
                            ps = psA.tile([128, T], f32, tag="pa")
                            for kc in range(DC):
                                mmr(nc, ps,
                                    wq_t[:, kc * D + mc * 128 : kc * D + (mc + 1) * 128],
                                    n1T[:, kc * PAIR : kc * PAIR + T],
                                    start=(kc == 0), stop=(kc == DC - 1))
                            eng = nc.scalar if mc % 2 == 0 else nc.vector
                            _cp(eng, qT[:, mc * T : (mc + 1) * T], ps)

                        # ---------- partner n1 arrives (gpsimd queue: waits on CC) ----------
                        for dc in range(DC):
                            nc.gpsimd.dma_start(
                                n1T[:, dc * PAIR + T : (dc + 1) * PAIR],
                                n1p[bass.ds(offp + dc * 128, 128), :],
                            )

                        # ---------- k, v over the pair ----------
                        kT = bigp.tile([128, DC * PAIR], bf16, tag="kT")
                        for mc in range(DC):
                            for nb in range(2):
                                ps = psA.tile([128, T], f32, tag="pa")
                                for kc in range(DC):
                                    mmr(nc, ps,
                                        wk_t[:, kc * D + mc * 128 : kc * D + (mc + 1) * 128],
                                        n1T[:, kc * PAIR + nb * T : kc * PAIR + (nb + 1) * T],
                                        start=(kc == 0), stop=(kc == DC - 1))
                                eng = nc.scalar if (mc + nb) % 2 == 0 else nc.vector
                                _cp(eng, kT[:, mc * PAIR + nb * T : mc * PAIR + nb * T + T], ps)
                        for pt in range(PAIR // 128):
                            ps = psA.tile([128, T], f32, tag="pa")
                            for kc in range(DC):
                                mmr(nc, ps,
                                    n1T[:, kc * PAIR + pt * 128 : kc * PAIR + (pt + 1) * 128],
                                    wv_t[:, kc * D : (kc + 1) * D],
                                    start=(kc == 0), stop=(kc == DC - 1))
                            vdst = v65[:, pt * 520 : (pt + 1) * 520].rearrange(
                                "p (h w) -> p h w", h=8, w=65
                            )[:, :, 0:64]
                            eng = nc.scalar if pt % 2 == 0 else nc.vector
                            _cp(eng, vdst, ps.rearrange("p (h w) -> p h w", h=8, w=64))

                        # ---------- attention ----------
                        ctxTu = medp.tile([128, DC * T], bf16, tag="ctxTu")
                        dctile = s1p.tile([64, TT * T], f32r, tag="dctile")
                        for h in range(H):
                            prow = (h % 2) * 64
                            cblk = h // 2
                            qTh = qT[prow : prow + 64, cblk * T : (cblk + 1) * T]
                            ctxps = psC.tile([65, T], f32, tag="pc")
                            for kt in range(8):
                                kTh = kT[
                                    prow : prow + 64,
                                    cblk * PAIR + kt * 128 : cblk * PAIR + (kt + 1) * 128,
                                ]
                                qs = kt * 128 if kt < 4 else 0
                                sps = psS.tile([128, T], f32, tag="ps")
                                mmr(nc, sps[:, qs:T], kTh, qTh[:, qs:T], start=True, stop=True)
                                ex = str3.tile([128, T], bf16, tag="ex")
                                bias = 0.0 if kt < 4 else biasg[:, 0:1]
                                nc.scalar.activation(
                                    ex[:, qs:T], sps[:, qs:T], AF.Exp,
                                    bias=bias, scale=1.0 / math.sqrt(DH),
                                )
                                if kt < 4:
                                    nc.vector.tensor_tensor(
                                        ex[:, kt * 128 : (kt + 1) * 128],
                                        ex[:, kt * 128 : (kt + 1) * 128],
                                        tri, op=ALU.mult,
                                    )
                                mmr(
                                    nc, ctxps[:, qs:T],
                                    v65[:, kt * 520 + h * 65 : kt * 520 + (h + 1) * 65],
                                    ex[:, qs:T],
                                    start=(kt == 0), stop=(kt == 7),
                                )
                            _cp(nc.scalar,
                                ctxTu[prow : prow + 64, cblk * T : (cblk + 1) * T],
                                ctxps[0:64, :])
                            _cp(nc.vector,
                                dctile[(h & 1) * 32 : (h & 1) * 32 + 1,
                                       (h // 2) * T : (h // 2) * T + T],
                                ctxps[64:65, :])
                        # broadcast den across partitions, then ctxTu /= den in place
                        for cblk in range(DC):
                            dvb = psS.tile([128, T], f32, tag="ps")
                            mmr(nc, dvb, selq,
                                dctile[:, cblk * T : (cblk + 1) * T],
                                start=True, stop=True)
                            ivs = strp.tile([128, T], f32r, tag="ivs")
                            with nc.allow_low_precision(reason="f32r reciprocal keeps full fp32 bits"):
                                nc.vector.reciprocal(ivs, dvb)
                            nc.vector.tensor_tensor(
                                ctxTu[:, cblk * T : (cblk + 1) * T],
                                ctxTu[:, cblk * T : (cblk + 1) * T],
                                ivs, op=ALU.mult,
                            )

                        # ---------- queryT = Ws.T @ [n1_own; ctx_norm] ----------
                        queryT = medp.tile([128, DC * T], bf16, tag="queryT")
                        for mc in range(DC):
                            ps = psA.tile([128, T], f32, tag="pa")
                            for kc in range(DC):
                                mmr(nc, ps,
                                    ws_t[:, kc * D + mc * 128 : kc * D + (mc + 1) * 128],
                                    n1T[:, kc * PAIR : kc * PAIR + T],
                                    start=(kc == 0), stop=False)
                            for kc in range(DC):
                                mmr(nc, ps,
                                    ws_t[:, (DC + kc) * D + mc * 128 : (DC + kc) * D + (mc + 1) * 128],
                                    ctxTu[:, kc * T : (kc + 1) * T],
                                    start=False, stop=(kc == DC - 1))
                            eng = nc.scalar if mc % 2 == 0 else nc.vector
                            _cp(eng, queryT[:, mc * T : (mc + 1) * T], ps)

                        # ---------- router scores + dense top-8 weights ----------
                        wdense = s1p.tile([128, TT * NNEU], f32, tag="wdense")
                        for tt in range(TT):
                            rps = psT.tile([128, 128], f32, tag="pt")
                            for dc in range(DC):
                                nc.tensor.matmul(
                                    rps[:, 0:NNEU],
                                    queryT[:, dc * T + tt * 128 : dc * T + (tt + 1) * 128],
                                    nembT[:, dc * NNEU : (dc + 1) * NNEU],
                                    start=(dc == 0), stop=(dc == DC - 1),
                                )
                            rsc = smallp.tile([128, NNEU], f32, tag="rsc")
                            nc.scalar.copy(rsc, rps[:, 0:NNEU])
                            top8 = smallp.tile([128, 8], f32, tag="top8")
                            nc.vector.max(top8, rsc)
                            e8 = smallp.tile([128, 8], f32, tag="e8")
                            s8 = smallp.tile([128, 1], f32, tag="s8")
                            nc.scalar.activation(e8, top8, AF.Exp, accum_out=s8)
                            inv8 = smallp.tile([128, 1], f32, tag="inv8")
                            nc.vector.reciprocal(inv8, s8)
                            wm = smallp.tile([128, NNEU], f32, tag="wm")
                            nc.vector.tensor_scalar(wm, rsc, top8[:, 7:8], None, op0=ALU.is_ge)
                            e64 = smallp.tile([128, NNEU], f32, tag="e64")
                            nc.scalar.activation(e64, rsc, AF.Exp)
                            nc.vector.scalar_tensor_tensor(
                                wdense[:, tt * NNEU : (tt + 1) * NNEU], e64, inv8, wm,
                                op0=ALU.mult, op1=ALU.mult,
                            )

                        # wT [64, T] f32, trT [32, T] bf16, tr [128, tt*32] bf16
                        wT = s1p.tile([64, T], f32, tag="wT")
                        for tt in range(TT):
                            wps = psT.tile([128, 128], f32, tag="pt")
                            nc.tensor.transpose(
                                wps[0:64, 0:128], wdense[:, tt * NNEU : (tt + 1) * NNEU], ident
                            )
                            nc.scalar.copy(wT[:, tt * 128 : (tt + 1) * 128], wps[0:64, 0:128])
                        trTb = s1p.tile([NB, T], bf16, tag="trTb")
                        trps = psS.tile([128, T], f32, tag="ps")
                        mmr(nc, trps[0:NB, :], pmat, wT, start=True, stop=True)
                        nc.scalar.copy(trTb, trps[0:NB, :])
                        tr = s1p.tile([128, TT * NB], bf16, tag="tr")
                        for tt in range(TT):
                            tps = psT.tile([128, 128], f32, tag="pt")
                            nc.tensor.matmul(
                                tps[:, 0:NB], wT[:, tt * 128 : (tt + 1) * 128], pmat,
                                start=True, stop=True,
                            )
                            nc.vector.tensor_copy(tr[:, tt * NB : (tt + 1) * NB], tps[:, 0:NB])

                        # ---------- tmat + h64: per-token weighted reduce over bases ----------
                        h64 = s1p.tile([128, TT * 64], f32, tag="h64")
                        for tt in range(TT):
                            tmat = tmpool.tile([128, NR], bf16, tag="tmat")
                            for nb4 in range(4):
                                ps = psA.tile([128, T], f32, tag="pa")
                                for dc in range(DC):
                                    mmr(nc, ps,
                                        n2T[:, dc * T + tt * 128 : dc * T + (tt + 1) * 128],
                                        af_bf[:, dc * NR + nb4 * 512 : dc * NR + (nb4 + 1) * 512],
                                        start=(dc == 0), stop=(dc == DC - 1))
                                eng = nc.scalar if nb4 % 2 == 0 else nc.vector
                                _cp(eng, tmat[:, nb4 * 512 : (nb4 + 1) * 512], ps)
                            # tmat cols are r-major: [r, n]; weight by tr over n, reduce n
                            nc.vector.tensor_tensor(
                                tmat.rearrange("p (r n) -> p r n", n=NB),
                                tmat.rearrange("p (r n) -> p r n", n=NB),
                                tr[:, tt * NB : (tt + 1) * NB].unsqueeze(1).to_broadcast(
                                    (128, RANK, NB)),
                                op=ALU.mult,
                            )
                            nc.vector.tensor_reduce(
                                h64[:, tt * 64 : (tt + 1) * 64],
                                tmat.rearrange("p (r n) -> p r n", n=NB),
                                axis=AX.X, op=ALU.add,
                            )
                        hT2 = s1p.tile([128, T], bf16, tag="hT2")
                        for tt in range(TT):
                            hps = psT.tile([128, 128], f32, tag="pt")
                            nc.tensor.transpose(
                                hps[0:64, 0:128], h64[:, tt * 64 : (tt + 1) * 64], ident
                            )
                            nc.scalar.copy(hT2[0:64, tt * 128 : (tt + 1) * 128], hps[0:64, 0:128])
                            nc.scalar.copy(hT2[64:128, tt * 128 : (tt + 1) * 128], hps[0:64, 0:128])

                        # ---------- uT (bf16) ----------
                        uT = bigp.tile([128, CC * T], bf16, tag="uT")
                        for c in range(CC):
                            rps = psS.tile([128, T], f32, tag="ps")
                            mmr(nc, rps, selb[:, c * 128 : (c + 1) * 128], trTb,
                                start=True, stop=True)
                            nc.vector.tensor_tensor(
                                uT[:, c * T : (c + 1) * T], rps, hT2, op=ALU.mult
                            )

                        # ---------- basis_B (bf16) + gelu -> hfg (bf16) ----------
                        hfg = bigp.tile([128, FC * T], bf16, tag="hfg")
                        for f in range(FC):
                            bt = bb2p.tile([128, NR], bf16, tag="bbrow")
                            nc.sync.dma_start(bt, bb_d[f, :, :])
                            hps = psS.tile([128, T], f32, tag="ps")
                            for c in range(CC):
                                nc.tensor.matmul(
                                    hps, bt[:, c * 128 : (c + 1) * 128],
                                    uT[:, c * T : (c + 1) * T],
                                    start=(c == 0), stop=(c == CC - 1),
                                )
                            nc.scalar.activation(hfg[:, f * T : (f + 1) * T], hps, gelu_af)

                        # ---------- Wd (bf16) + residual (d-major, no transpose) ----------
                        for dc in range(DC):
                            fps = psA.tile([128, T], f32, tag="pa")
                            for f in range(FC):
                                nc.tensor.matmul(
                                    fps, wd_t[:, f * D + dc * 128 : f * D + (dc + 1) * 128],
                                    hfg[:, f * T : (f + 1) * T],
                                    start=(f == 0), stop=(f == FC - 1),
                                )
                            nc.vector.tensor_tensor(
                                xl[:, dc * T : (dc + 1) * T],
                                xl[:, dc * T : (dc + 1) * T],
                                fps, op=ALU.add,
                            )

                    # ---------- final LN (reuses layer pools) + 8-way AllGather ----------
                    layer_norm_T(lambda dc: xl[:, dc * T : (dc + 1) * T], xfT, T)
                    xfb = dpool.tile([D, T], bf16, tag="xfb")
                    nc.sync.dma_start(
                        xfb.rearrange("(c p) t -> p c t", p=128),
                        xfT.rearrange("p (c t) -> p c t", c=DC),
                    )
                    xfall = dpool.tile([NC * D, T], bf16, tag="xfall", addr_space="Shared")
                    nc.gpsimd.collective_compute(
                        "AllGather",
                        ALU.bypass,
                        replica_groups=[list(range(NC))],
                        ins=[xfb.opt()],
                        outs=[xfall.opt()],
                    )

                # ---------- head ----------
                with ExitStack() as _st2:
                    hxp = _st2.enter_context(tc.tile_pool(name="hx", bufs=1))
                    hrp = _st2.enter_context(tc.tile_pool(name="hr", bufs=2))
                    hstp = _st2.enter_context(tc.tile_pool(name="hst", bufs=4))
                    php = _st2.enter_context(tc.tile_pool(name="ph", bufs=8, space="PSUM"))
                    ect_bf = hxp.tile([128, DC * VSH], bf16, tag="ect")
                    for dc in range(DC):
                        nc.sync.dma_start(
                            ect_bf[:, dc * VSH : (dc + 1) * VSH],
                            ect_d[dc * 128 : (dc + 1) * 128, :],
                        )
                    xr_next = None
                    for j in range(NC):
                        src = xfT if j == 0 else xr_next
                        if j + 1 < NC:
                            nxt = hrp.tile([128, DC * T], bf16, tag="xr")
                            roff = ((pid + j + 1) & 7) * D
                            for dc in range(DC):
                                nc.gpsimd.dma_start(
                                    nxt[:, dc * T : (dc + 1) * T],
                                    xfall[bass.ds(roff + dc * 128, 128), :],
                                )
                            xr_next = nxt
                        rowb = ((pid + j) & 7) * T
                        for tt in range(TT):
                            osb = hstp.tile([128, VSH], fp16, tag="osb")
                            for g in range(2):
                                pss = []
                                for _pi in range(4):
                                    psx = php.tile([128, 512], f32, tag="ph")
                                    pss.append(psx)
                                for dc in range(DC):
                                    lhs = src[:, dc * T + tt * 128 : dc * T + (tt + 1) * 128]
                                    for i in range(4):
                                        vb = 4 * g + i
                                        w = VWID[vb]
                                        nc.tensor.matmul(
                                            pss[i][:, 0:w], lhs,
                                            ect_bf[:, dc * VSH + vb * 512 : dc * VSH + vb * 512 + w],
                                            start=(dc == 0), stop=(dc == DC - 1),
                                        )
                                for i in range(4):
                                    vb = 4 * g + i
                                    w = VWID[vb]
                                    eng = nc.scalar if vb % 2 == 0 else nc.vector
                                    _cp(eng, osb[:, vb * 512 : vb * 512 + w], pss[i][:, 0:w])
                            nc.sync.dma_start(
                                out_d[bass.ds(rowb + tt * 128, 128), :], osb
                            )
    nc.compile()
    return nc


# ============================ host side ============================

_EXEC = {}


def _prep_in_maps(inputs):
    import ml_dtypes

    ids = np.asarray(inputs["input_ids"]).astype(np.int64).reshape(B, S)
    emb = np.asarray(inputs["token_emb"], dtype=np.float32)
    pos = np.asarray(inputs["pos_emb"], dtype=np.float32)
    bf = ml_dtypes.bfloat16
    # Abf[d, r*NB + n] = basis_A[n, d, r]
    Abf = np.ascontiguousarray(
        np.asarray(inputs["basis_A"], dtype=np.float32).transpose(1, 2, 0).reshape(D, NR)
    ).astype(bf)
    BBf = np.asarray(inputs["basis_B"], dtype=np.float32).reshape(NR, DFF)
    # BBbf[f, p, c*128+j] = BBf[c*128+p, f*128+j]
    BBbf = np.ascontiguousarray(
        BBf.reshape(CC, 128, FC, 128).transpose(2, 1, 0, 3).reshape(FC, 128, NR)
    ).astype(bf)
    selb = np.zeros((NB, CC * 128), np.float32)
    for c in range(CC):
        for p in range(128):
            selb[2 * c + p // 64, c * 128 + p] = 1.0
    selq = np.zeros((64, 128), np.float32)
    selq[0, 0:64] = 1.0
    selq[32, 64:128] = 1.0
    ET = np.ascontiguousarray(emb.T)
    shared = {
        "Wqb": np.ascontiguousarray(np.asarray(inputs["Wq"], dtype=np.float32)).astype(bf),
        "Wkb": np.ascontiguousarray(np.asarray(inputs["Wk"], dtype=np.float32)).astype(bf),
        "Wvb": np.ascontiguousarray(np.asarray(inputs["Wv"], dtype=np.float32)).astype(bf),
        "Wsb": np.ascontiguousarray(np.asarray(inputs["Ws"], dtype=np.float32)).astype(bf),
        "Wdbf": np.ascontiguousarray(np.asarray(inputs["Wd"], dtype=np.float32)).astype(bf),
        "Abf": Abf,
        "BBbf": BBbf,
        "recipe": np.ascontiguousarray(np.asarray(inputs["recipe"], dtype=np.float32)),
        "bemb": np.ascontiguousarray(np.asarray(inputs["basis_emb"], dtype=np.float32)),
        "ident": np.eye(128, dtype=np.float32),
        "trib": np.triu(np.ones((128, 128), np.float32)).astype(bf),
        "selb": selb.astype(bf),
        "selq": selq,
        "onescol": np.ones((128, 1), np.float32),
        "onesrow": np.ones((1, 128), np.float32),
    }
    in_maps = []
    for c in range(NC):
        b, hh = c // 2, c % 2
        ids_c = ids[b, hh * T : (hh + 1) * T]
        x0 = emb[ids_c] + pos[hh * T : (hh + 1) * T]
        m = dict(shared)
        m["x0T"] = np.ascontiguousarray(x0.T, dtype=np.float32)
        m["EcTb"] = np.ascontiguousarray(ET[:, c * VSH : (c + 1) * VSH]).astype(bf)
        m["biasg"] = np.full((128, 1), 0.0 if hh == 1 else NEG, np.float32)
        in_maps.append(m)
    return in_maps


def _get_exec(reps=1):
    if reps in _EXEC:
        return _EXEC[reps]
    import jax
    import jax.numpy as jnp
    from jax.experimental.shard_map import shard_map
    from jax.sharding import Mesh, PartitionSpec, NamedSharding
    from concourse import bass2jax

    nc = build_nc(reps=reps)
    bass2jax.install_neuronx_cc_hook()
    partition_name = nc.partition_id_tensor.name if nc.partition_id_tensor else None

    in_names, out_names, out_avals, zero_shapes = [], [], [], []
    for alloc in nc.m.functions[0].allocations:
        if not isinstance(alloc, mybir.MemoryLocationSet):
            continue
        name = alloc.memorylocations[0].name
        if alloc.kind == "ExternalInput":
            if name != partition_name:
                in_names.append(name)
        elif alloc.kind == "ExternalOutput":
            shape = tuple(alloc.tensor_shape)
            dtype = mybir.dt.np(alloc.dtype)
            out_names.append(name)
            out_avals.append(jax.core.ShapedArray(shape, dtype))
            zero_shapes.append((shape, dtype))
    n_params = len(in_names)
    n_outs = len(out_names)
    all_names = list(in_names) + list(out_names)
    if partition_name is not None:
        all_names.append(partition_name)

    def _body(*args):
        operands = list(args)
        if partition_name is not None:
            operands.append(bass2jax.partition_id_tensor())
        outs = bass2jax._bass_exec_p.bind(
            *operands,
            out_avals=tuple(out_avals),
            in_names=tuple(all_names),
            out_names=tuple(out_names),
            lowering_input_output_aliases=(),
            sim_require_finite=False,
            sim_require_nnan=False,
            nc=nc,
        )
        return tuple(outs)

    devices = jax.devices()[:NC]
    mesh = Mesh(np.asarray(devices), ("core",))
    spec = PartitionSpec("core")
    sharded = jax.jit(
        shard_map(
            _body, mesh=mesh,
            in_specs=(spec,) * (n_params + n_outs),
            out_specs=(spec,) * n_outs,
            check_rep=False,
        ),
        donate_argnums=tuple(range(n_params, n_params + n_outs)),
        keep_unused=True,
    )
    zfn = jax.jit(
        shard_map(
            lambda: tuple(jnp.zeros(s, d) for s, d in zero_shapes),
            mesh=mesh, in_specs=(), out_specs=(spec,) * n_outs, check_rep=False,
        )
    )
    state = {
        "sharded": sharded, "zfn": zfn, "mesh": mesh, "spec": spec,
        "in_names": in_names, "out_names": out_names, "out_avals": out_avals,
        "jax": jax, "NamedSharding": NamedSharding,
    }
    _EXEC[reps] = state
    return state


def _device_inputs(state, in_maps):
    jax = state["jax"]
    sh = state["NamedSharding"](state["mesh"], state["spec"])
    dev = []
    for nm in state["in_names"]:
        arr = np.concatenate([np.asarray(in_maps[c][nm]) for c in range(NC)], axis=0)
        dev.append(jax.device_put(arr, sh))
    return dev


def _run_once(state, dev_in):
    zeros = state["zfn"]()
    out_arrs = state["sharded"](*dev_in, *zeros)
    return out_arrs


def _to_results(state, out_arrs):
    return [
        {
            nm: np.asarray(out_arrs[i]).reshape(NC, *state["out_avals"][i].shape)[c]
            for i, nm in enumerate(state["out_names"])
        }
        for c in range(NC)
    ]


def _assemble(results):
    parts = [results[c]["logits"] for c in range(NC)]
    full = np.concatenate(parts, axis=1)
    return full.reshape(B, S, VOCAB).astype(np.float32)


def kernel(**inputs):
    state = _get_exec()
    in_maps = _prep_in_maps(inputs)
    dev_in = _device_inputs(state, in_maps)
    out_arrs = _run_once(state, dev_in)
    return _assemble(_to_results(state, out_arrs))


def _time_state(state, dev_in, iters):
    import time

    res = _run_once(state, dev_in)  # warm
    for a in res:
        a.block_until_ready()
    times = []
    for _ in range(iters):
        t0 = time.perf_counter()
        res = _run_once(state, dev_in)
        for a in res:
            a.block_until_ready()
        times.append(time.perf_counter() - t0)
    return min(times), res


def bench(iters=8, reps=8, **inputs):
    """Returns (output, est. device seconds per kernel run) via reps differencing."""
    in_maps = _prep_in_maps(inputs)
    s1 = _get_exec(1)
    d1 = _device_inputs(s1, in_maps)
    t1, res = _time_state(s1, d1, iters)
    sN = _get_exec(reps)
    dN = _device_inputs(sN, in_maps)
    tN, _ = _time_state(sN, dN, iters)
    dev = (tN - t1) / (reps - 1)
    print(f"[bench] wall reps=1: {t1*1e3:.2f} ms, reps={reps}: {tN*1e3:.2f} ms -> device {dev*1e6:.0f} us")
    return _assemble(_to_results(s1, res)), dev


def profile_exec_ns(inputs, tmpdir=None, trace_cores=(0,)):
    """Run once on hardware with NTFF profiling; returns (output, exec_time_ns)."""
    from concourse import bass_utils

    in_maps = _prep_in_maps(inputs)
    nc = build_nc(reps=1)
    res = bass_utils.run_bass_kernel_spmd(
        nc, in_maps, core_ids=list(range(NC)),
        trace=True, tmpdir=tmpdir, trace_cores=list(trace_cores),
    )
    out = _assemble(res.results)
    ns = res.exec_time_ns
    if res.mean_exec_time_ns is not None:
        ns = max(ns or 0, int(res.mean_exec_time_ns))
    return out, ns
